# revision 3
# baseline (speedup 1.0000x reference)
"""BitConv2d Trainium2 kernel.

Math: the reference decomposes integer-valued x (in [0, 2^8)) into 8 scaled
bit planes, convolves each plane with W, and sums. Since the planes sum back
to x exactly (n_scale=1) and convolution is linear, the whole module equals

    y = conv2d(x, W, pad=1) + bias

Implementation: data-parallel over batch across 8 NeuronCores (2 images per
core). Each core computes a direct convolution as 9 accumulating 128x128
matmuls per output tile (contraction over C_in=128 on the partition dim,
one matmul per 3x3 tap position), free dim = 8 output rows x 56 cols = 448.
Inputs are fed in fp16: x values are small integers (exact in fp16) and W's
fp16 rounding (2^-11) keeps the result ~1e-4 relative error, far inside the
gate, while running the PE at full (1 cycle/row) speed.
"""

import numpy as np

import concourse.bass as bass
import concourse.mybir as mybir
import concourse.tile as tile
from concourse import bacc
from concourse.bass_utils import run_bass_kernel_spmd

# Problem shapes (hardcoded per harness contract)
B, C, H, W_ = 16, 128, 56, 56
O = 128
KH = KW = 3
N_CORES = 8
BPC = B // N_CORES          # images per core
HP, WP = H + 2, W_ + 2      # zero-padded input dims
ROWS = 8                    # output rows per matmul tile
CHUNKS = (BPC * H) // ROWS  # 14 output tiles per core
WARMUP_MM = 40              # PE warmup matmuls issued while input DMA runs

_CACHE = {}


def _build_nc():
    nc = bacc.Bacc("TRN2", target_bir_lowering=False, debug=False)

    x_d = nc.dram_tensor("x", [C, BPC, HP, WP], mybir.dt.float16, kind="ExternalInput")
    w_d = nc.dram_tensor("w", [C, KH * KW, O], mybir.dt.float16, kind="ExternalInput")
    b_d = nc.dram_tensor("b", [O, 1], mybir.dt.float32, kind="ExternalInput")
    y_d = nc.dram_tensor("y", [O, BPC, H, W_], mybir.dt.float32, kind="ExternalOutput")

    with tile.TileContext(nc) as tc:
        with (
            tc.tile_pool(name="const", bufs=1) as cpool,
            tc.tile_pool(name="xin", bufs=1) as xpool,
            tc.tile_pool(name="outs", bufs=4) as opool,
            tc.tile_pool(name="psum", bufs=4, space="PSUM") as ppool,
        ):
            # PE warmup: keep TensorE busy during the input DMA so HAM is at
            # 2.4 GHz when the real matmuls start.
            warm = cpool.tile([128, 128], mybir.dt.float16)
            nc.vector.memset(warm[:], 0.0)
            warm_ps = ppool.tile([128, 128], mybir.dt.float32, tag="warm", bufs=1)
            for _ in range(WARMUP_MM):
                nc.tensor.matmul(warm_ps[:], warm[:], warm[:], start=True, stop=True)

            x_sb = xpool.tile([C, BPC, HP, WP], mybir.dt.float16)
            w_sb = cpool.tile([C, KH * KW, O], mybir.dt.float16)
            b_sb = cpool.tile([O, 1], mybir.dt.float32)
            nc.sync.dma_start(x_sb[:], x_d[:])
            nc.sync.dma_start(w_sb[:], w_d[:])
            nc.sync.dma_start(b_sb[:], b_d[:])

            for ci in range(CHUNKS):
                img, r0 = divmod(ci * ROWS, H)
                ps = ppool.tile([O, ROWS, W_], mybir.dt.float32, tag="ps")
                for k in range(KH * KW):
                    kh, kw = divmod(k, KW)
                    rhs = x_sb[:, img, r0 + kh : r0 + kh + ROWS, kw : kw + W_]
                    nc.tensor.matmul(
                        ps[:], w_sb[:, k, :], rhs,
                        start=(k == 0), stop=(k == KH * KW - 1),
                    )
                ot = opool.tile([O, ROWS, W_], mybir.dt.float32)
                nc.vector.tensor_scalar_add(out=ot[:], in0=ps[:], scalar1=b_sb[:])
                nc.sync.dma_start(y_d[:, img, r0 : r0 + ROWS, :], ot[:])

    nc.compile()
    return nc


def _get_nc():
    if "nc" not in _CACHE:
        _CACHE["nc"] = _build_nc()
    return _CACHE["nc"]


def _prep_in_maps(x, W, bias):
    # Zero-pad H/W and cast to fp16 (exact: x holds integers < 2^11).
    xp = np.zeros((B, C, HP, WP), np.float16)
    xp[:, :, 1 : H + 1, 1 : W_ + 1] = x
    # lhsT layout: [K=C_in, tap, M=C_out]
    wt = np.ascontiguousarray(
        W.transpose(1, 2, 3, 0).reshape(C, KH * KW, O).astype(np.float16)
    )
    bt = np.ascontiguousarray(bias.reshape(O, 1).astype(np.float32))
    in_maps = []
    for i in range(N_CORES):
        xs = np.ascontiguousarray(
            xp[i * BPC : (i + 1) * BPC].transpose(1, 0, 2, 3)
        )  # [C, BPC, HP, WP]
        in_maps.append({"x": xs, "w": wt, "b": bt})
    return in_maps


def kernel(x, W, bias, _trace=False, _trace_kwargs=None):
    nc = _get_nc()
    in_maps = _prep_in_maps(
        np.asarray(x, np.float32), np.asarray(W, np.float32),
        np.asarray(bias, np.float32),
    )
    res = run_bass_kernel_spmd(
        nc, in_maps, list(range(N_CORES)),
        trace=_trace, **(_trace_kwargs or {}),
    )
    y = np.stack([r["y"] for r in res.results])        # [8, O, BPC, H, W]
    y = y.transpose(0, 2, 1, 3, 4).reshape(B, O, H, W_)
    if _trace:
        return np.ascontiguousarray(y), res
    return np.ascontiguousarray(y)


# revision 5
# speedup vs baseline: 1.0906x; 1.0906x over previous
"""BitConv2d Trainium2 kernel.

Math: the reference decomposes integer-valued x (in [0, 2^8)) into 8 scaled
bit planes, convolves each plane with W, and sums. Since the planes sum back
to x exactly (n_scale=1) and convolution is linear, the whole module equals

    y = conv2d(x, W, pad=1) + bias

Implementation: data-parallel over batch across 8 NeuronCores (2 images per
core). Each core computes a direct convolution as 9 accumulating 128x128
matmuls per output tile (contraction over C_in=128 on the partition dim,
one matmul per 3x3 tap position), free dim = 8 output rows x 56 cols = 448.
Inputs are fed in fp16: x values are small integers (exact in fp16) and W's
fp16 rounding (2^-11) keeps the result ~1e-4 relative error, far inside the
gate, while running the PE at full (1 cycle/row) speed.
"""

import numpy as np

import concourse.bass as bass
import concourse.mybir as mybir
import concourse.tile as tile
from concourse import bacc
from concourse.bass_utils import run_bass_kernel_spmd

# Problem shapes (hardcoded per harness contract)
B, C, H, W_ = 16, 128, 56, 56
O = 128
KH = KW = 3
N_CORES = 8
BPC = B // N_CORES          # images per core
HP, WP = H + 2, W_ + 2      # zero-padded input dims
ROWS = 8                    # output rows per matmul tile
CHUNKS = (BPC * H) // ROWS  # 14 output tiles per core
WARMUP_MM = 5               # PE warmup matmuls issued while input DMA runs
X_SPLIT = 30                # padded-row boundary for split input DMAs

_CACHE = {}


def _build_nc():
    nc = bacc.Bacc("TRN2", target_bir_lowering=False, debug=False)

    x_d = nc.dram_tensor("x", [C, BPC, HP, WP], mybir.dt.float16, kind="ExternalInput")
    w_d = nc.dram_tensor("w", [C, KH * KW, O], mybir.dt.float16, kind="ExternalInput")
    b_d = nc.dram_tensor("b", [O, 1], mybir.dt.float32, kind="ExternalInput")
    y_d = nc.dram_tensor("y", [O, BPC, H, W_], mybir.dt.float32, kind="ExternalOutput")

    with tile.TileContext(nc) as tc:
        with (
            tc.tile_pool(name="const", bufs=1) as cpool,
            tc.tile_pool(name="xin", bufs=1) as xpool,
            tc.tile_pool(name="outs", bufs=4) as opool,
            tc.tile_pool(name="psum", bufs=4, space="PSUM") as ppool,
        ):
            # PE warmup: keep TensorE busy during the input DMA so HAM is at
            # 2.4 GHz when the real matmuls start.
            warm = cpool.tile([128, 512], mybir.dt.float16)
            nc.vector.memset(warm[:], 0.0)
            warm_ps = ppool.tile([128, 512], mybir.dt.float32, tag="warm", bufs=1)
            for _ in range(WARMUP_MM):
                nc.tensor.matmul(
                    warm_ps[:], warm[:, :128], warm[:], start=True, stop=True
                )

            x_sb = xpool.tile([C, BPC, HP, WP], mybir.dt.float16)
            w_sb = cpool.tile([C, KH * KW, O], mybir.dt.float16)
            b_sb = cpool.tile([O, 1], mybir.dt.float32)
            # Split the x transfer so the first matmuls start as soon as the
            # first piece lands; inputs ride the Scalar HWDGE ring so their
            # descriptor generation runs parallel to the Sync ring used for
            # outputs. W goes on Sync (needed by the first matmul too).
            nc.sync.dma_start(w_sb[:], w_d[:])
            for b in range(BPC):
                nc.scalar.dma_start(
                    x_sb[:, b, :X_SPLIT, :], x_d[:, b, :X_SPLIT, :]
                )
                nc.scalar.dma_start(
                    x_sb[:, b, X_SPLIT:, :], x_d[:, b, X_SPLIT:, :]
                )
            nc.sync.dma_start(b_sb[:], b_d[:])

            for ci in range(CHUNKS):
                img, r0 = divmod(ci * ROWS, H)
                ps = ppool.tile([O, ROWS, W_], mybir.dt.float32, tag="ps")
                for k in range(KH * KW):
                    kh, kw = divmod(k, KW)
                    rhs = x_sb[:, img, r0 + kh : r0 + kh + ROWS, kw : kw + W_]
                    nc.tensor.matmul(
                        ps[:], w_sb[:, k, :], rhs,
                        start=(k == 0), stop=(k == KH * KW - 1),
                    )
                ot = opool.tile([O, ROWS, W_], mybir.dt.float32)
                nc.vector.tensor_scalar_add(out=ot[:], in0=ps[:], scalar1=b_sb[:])
                nc.sync.dma_start(y_d[:, img, r0 : r0 + ROWS, :], ot[:])

    nc.compile()
    return nc


def _get_nc():
    if "nc" not in _CACHE:
        _CACHE["nc"] = _build_nc()
    return _CACHE["nc"]


def _prep_in_maps(x, W, bias):
    # Zero-pad H/W and cast to fp16 (exact: x holds integers < 2^11).
    xp = np.zeros((B, C, HP, WP), np.float16)
    xp[:, :, 1 : H + 1, 1 : W_ + 1] = x
    # lhsT layout: [K=C_in, tap, M=C_out]
    wt = np.ascontiguousarray(
        W.transpose(1, 2, 3, 0).reshape(C, KH * KW, O).astype(np.float16)
    )
    bt = np.ascontiguousarray(bias.reshape(O, 1).astype(np.float32))
    in_maps = []
    for i in range(N_CORES):
        xs = np.ascontiguousarray(
            xp[i * BPC : (i + 1) * BPC].transpose(1, 0, 2, 3)
        )  # [C, BPC, HP, WP]
        in_maps.append({"x": xs, "w": wt, "b": bt})
    return in_maps


def kernel(x, W, bias, _trace=False, _trace_kwargs=None):
    nc = _get_nc()
    in_maps = _prep_in_maps(
        np.asarray(x, np.float32), np.asarray(W, np.float32),
        np.asarray(bias, np.float32),
    )
    res = run_bass_kernel_spmd(
        nc, in_maps, list(range(N_CORES)),
        trace=_trace, **(_trace_kwargs or {}),
    )
    y = np.stack([r["y"] for r in res.results])        # [8, O, BPC, H, W]
    y = y.transpose(0, 2, 1, 3, 4).reshape(B, O, H, W_)
    if _trace:
        return np.ascontiguousarray(y), res
    return np.ascontiguousarray(y)


# revision 8
# speedup vs baseline: 1.1551x; 1.0592x over previous
"""BitConv2d Trainium2 kernel.

Math: the reference decomposes integer-valued x (in [0, 2^8)) into 8 scaled
bit planes, convolves each plane with W, and sums. Since the planes sum back
to x exactly (n_scale=1) and convolution is linear, the whole module equals

    y = conv2d(x, W, pad=1) + bias

Implementation: data-parallel over batch across 8 NeuronCores (2 images per
core). Each core computes a direct convolution as 9 accumulating 128x128
matmuls per output tile (contraction over C_in=128 on the partition dim,
one matmul per 3x3 tap position), free dim = 8 output rows x 56 cols = 448.
Inputs are fed in fp16: x values are small integers (exact in fp16) and W's
fp16 rounding (2^-11) keeps the result ~1e-4 relative error, far inside the
gate, while running the PE at full (1 cycle/row) speed.
"""

import numpy as np

import concourse.bass as bass
import concourse.mybir as mybir
import concourse.tile as tile
from concourse import bacc
from concourse.bass_utils import run_bass_kernel_spmd

# Problem shapes (hardcoded per harness contract)
B, C, H, W_ = 16, 128, 56, 56
O = 128
KH = KW = 3
N_CORES = 8
BPC = B // N_CORES          # images per core
HP, WP = H + 2, W_ + 2      # zero-padded input dims
ROWS = 8                    # output rows per matmul tile
WARMUP_MM = 3               # PE warmup matmuls issued while input DMA runs
X_PIECES = (0, 12, 34, HP)  # padded-row boundaries for split input DMAs

_CACHE = {}


def _build_nc():
    nc = bacc.Bacc("TRN2", target_bir_lowering=False, debug=False)

    x_d = nc.dram_tensor("x", [C, BPC, HP, WP], mybir.dt.float16, kind="ExternalInput")
    w_d = nc.dram_tensor("w", [C, KH * KW, O], mybir.dt.float16, kind="ExternalInput")
    b_d = nc.dram_tensor("b", [O, 1], mybir.dt.float32, kind="ExternalInput")
    y_d = nc.dram_tensor("y", [O, BPC, H, W_], mybir.dt.float32, kind="ExternalOutput")

    with tile.TileContext(nc) as tc:
        with (
            tc.tile_pool(name="const", bufs=1) as cpool,
            tc.tile_pool(name="xin", bufs=1) as xpool,
            tc.tile_pool(name="outs", bufs=4) as opool,
            tc.tile_pool(name="psum", bufs=4, space="PSUM") as ppool,
        ):
            # PE warmup: keep TensorE busy during the input DMA so HAM is at
            # 2.4 GHz when the real matmuls start.
            warm = cpool.tile([128, 512], mybir.dt.float16)
            nc.vector.memset(warm[:], 0.0)
            warm_ps = ppool.tile([128, 512], mybir.dt.float32, tag="warm", bufs=1)
            for _ in range(WARMUP_MM):
                nc.tensor.matmul(
                    warm_ps[:], warm[:, :128], warm[:], start=True, stop=True
                )

            x_sb = xpool.tile([C, BPC, HP, WP], mybir.dt.float16)
            w_sb = cpool.tile([C, KH * KW, O], mybir.dt.float16)
            b_sb = cpool.tile([O, 1], mybir.dt.float32)
            # Split the x transfer so the first matmuls start as soon as the
            # first piece lands. Descriptor generation is serialized per
            # HWDGE ring, so image 0 pieces ride the Scalar ring while W and
            # image 1 pieces ride the Sync ring (parallel desc-gen).
            nc.sync.dma_start(w_sb[:], w_d[:])
            for r0, r1 in zip(X_PIECES[:-1], X_PIECES[1:]):
                nc.scalar.dma_start(x_sb[:, 0, r0:r1, :], x_d[:, 0, r0:r1, :])
            for r0, r1 in zip(X_PIECES[:-1], X_PIECES[1:]):
                nc.sync.dma_start(x_sb[:, 1, r0:r1, :], x_d[:, 1, r0:r1, :])
            nc.scalar.dma_start(b_sb[:], b_d[:])

            # Output tiles: 8-row chunks, except the final chunk is split
            # into two 4-row groups so its eviction + store overlap the last
            # matmuls instead of sitting fully exposed on the tail.
            tiles = []
            for ci in range(BPC * H // ROWS):
                img, r0 = divmod(ci * ROWS, H)
                tiles.append((img, r0, ROWS))
            img, r0, _ = tiles.pop()
            tiles.append((img, r0, ROWS // 2))
            tiles.append((img, r0 + ROWS // 2, ROWS // 2))

            for ti, (img, r0, nrows) in enumerate(tiles):
                ps = ppool.tile([O, ROWS, W_], mybir.dt.float32, tag="ps")
                for k in range(KH * KW):
                    kh, kw = divmod(k, KW)
                    rhs = x_sb[:, img, r0 + kh : r0 + kh + nrows, kw : kw + W_]
                    nc.tensor.matmul(
                        ps[:, :nrows, :], w_sb[:, k, :], rhs,
                        start=(k == 0), stop=(k == KH * KW - 1),
                    )
                ot = opool.tile([O, ROWS, W_], mybir.dt.float32)
                nc.vector.tensor_scalar_add(
                    out=ot[:, :nrows, :], in0=ps[:, :nrows, :], scalar1=b_sb[:]
                )
                eng = nc.scalar if ti == len(tiles) - 1 else nc.sync
                eng.dma_start(y_d[:, img, r0 : r0 + nrows, :], ot[:, :nrows, :])

    nc.compile()
    return nc


def _get_nc():
    if "nc" not in _CACHE:
        _CACHE["nc"] = _build_nc()
    return _CACHE["nc"]


def _prep_in_maps(x, W, bias):
    # Zero-pad H/W and cast to fp16 (exact: x holds integers < 2^11).
    xp = np.zeros((B, C, HP, WP), np.float16)
    xp[:, :, 1 : H + 1, 1 : W_ + 1] = x
    # lhsT layout: [K=C_in, tap, M=C_out]
    wt = np.ascontiguousarray(
        W.transpose(1, 2, 3, 0).reshape(C, KH * KW, O).astype(np.float16)
    )
    bt = np.ascontiguousarray(bias.reshape(O, 1).astype(np.float32))
    in_maps = []
    for i in range(N_CORES):
        xs = np.ascontiguousarray(
            xp[i * BPC : (i + 1) * BPC].transpose(1, 0, 2, 3)
        )  # [C, BPC, HP, WP]
        in_maps.append({"x": xs, "w": wt, "b": bt})
    return in_maps


def kernel(x, W, bias, _trace=False, _trace_kwargs=None):
    nc = _get_nc()
    in_maps = _prep_in_maps(
        np.asarray(x, np.float32), np.asarray(W, np.float32),
        np.asarray(bias, np.float32),
    )
    res = run_bass_kernel_spmd(
        nc, in_maps, list(range(N_CORES)),
        trace=_trace, **(_trace_kwargs or {}),
    )
    y = np.stack([r["y"] for r in res.results])        # [8, O, BPC, H, W]
    y = y.transpose(0, 2, 1, 3, 4).reshape(B, O, H, W_)
    if _trace:
        return np.ascontiguousarray(y), res
    return np.ascontiguousarray(y)
